# revision 2
# baseline (speedup 1.0000x reference)
"""Trainium2 Bass kernel for nn_BlurTensor: gaussian_filter(sigma=k_size) over
ALL axes of x (B=32, C=3, H=512, W=512) with 'symmetric' boundary.

Decomposition: the blur is the linear operator A0 (x) A1 (x) A2 (x) A3 applied
as mode products (one blur matrix per axis, built on host from k_size).
B and C fold into a single 96x96 Kronecker-product mixing matrix (96 <= 128
partitions), so the device does three matmul passes: H, W (banded), BC.

Sharding: H axis split into 8 x 64-row output slabs; each core receives a
104-row input slab (radius-20 halo), which makes all passes core-local.

v2 (all-fp16 device pipeline, PSUM accumulation stays fp32):
  - fp16 matmuls stream 1 cycle/row vs fp32's 4 (fp32 = 2 half-speed passes);
    fp16 input halves the dominant DMA traffic; device output is fp16 too
    (host upcasts) halving output DMA. End-to-end rel err ~6e-4 vs the 2e-2
    gate.
  - One merged consts tensor (fewer per-call PJRT args); x pre-transposed on
    host to [hin, B*C, W] so every input DMA line is contiguous.
  - PSUM evacuations paired (two banks per copy) to halve the per-op DVE/ACT
    fixed cost; copies alternate between Vector and Scalar engines.

Device pipeline per core (all intermediates SBUF-resident):
  pass H : out[w, (bc,h)] via lhsT = data tile [h'=104, w-chunk 128],
           rhs = A2_local^T [104, 64]  -> X1 [P:w(4x128), F:(wc,bc,h)]
  pass W : banded 512x512 matrix, 4 chunked matmuls accumulating into one
           PSUM bank per h (start=True on first clears has_written; the
           overlapping band writes then accumulate-or-overwrite per element)
  pass BC: Kronecker matrix (padded to 128 cols for fast-weight-load) as
           stationary weights, rhs = X2h [96, 512]
"""

import numpy as np

import concourse.bass as bass
import concourse.bacc as bacc
import concourse.mybir as mybir
from concourse.tile import TileContext
from concourse.bass_utils import run_bass_kernel_spmd

TRUNCATE = 4.0
N_CORES = 8
F32 = mybir.dt.float32
DEV_DT = mybir.dt.float16
DEV_NP = np.float16


def _gauss_kernel(sigma):
    # matches scipy/jax _gaussian_kernel1d in float32, like the reference
    radius = int(TRUNCATE * sigma + 0.5)
    x = np.arange(-radius, radius + 1, dtype=np.float32)
    w = np.exp(-0.5 * (x / sigma) ** 2).astype(np.float32)
    w = (w / w.sum(dtype=np.float32)).astype(np.float32)
    return w.astype(np.float64), radius


def _blur_matrix(L, w, radius):
    """(L, L) float64 operator: blur(v) = A @ v with symmetric padding."""
    I = np.eye(L, dtype=np.float64)
    Ipad = np.pad(I, ((radius, radius), (0, 0)), mode="symmetric")
    A = np.empty((L, L), dtype=np.float64)
    for i in range(L):
        A[i, :] = w @ Ipad[i : i + 2 * radius + 1, :]
    return A


def _build_program(B, C, H, W, hin, hs, radius, n_reps=1):
    """Build the SPMD Bass program (identical on all cores).

    n_reps > 1 wraps the whole pipeline in a device-side For_i that re-runs
    it n_reps times (same DRAM buffers). Used only for timing: the axon PJRT
    dispatch costs ~3.3 ms per call, so per-exec HW time is measured as the
    slope between two rep counts."""
    from contextlib import ExitStack

    BC = B * C
    assert BC <= 128 and hin <= 128 and W % 128 == 0
    NJ = W // 128  # w' chunks
    BCP = 128  # mbct padded cols (bc_out) for fast-weight-load

    n_c = hin * hs + 128 * NJ * W + BC * BCP

    nc = bacc.Bacc("TRN2")
    xs = nc.dram_tensor("xs", [hin, BC, W], DEV_DT, kind="ExternalInput")
    consts = nc.dram_tensor("consts", [n_c], DEV_DT, kind="ExternalInput")
    out = nc.dram_tensor("out", [BC, hs, W], DEV_DT, kind="ExternalOutput")

    GRP = 8  # bc per psum/copy group
    LDG = 2  # load groups per DMA
    n_grp = BC // GRP
    HB = 8  # h rows per output stage/DMA

    def band(j):
        return max(0, 128 * j - radius), min(W, 128 * j + 128 + radius)

    with TileContext(nc) as tc, ExitStack() as _st:
        if n_reps > 1:
            _st.enter_context(
                tc.For_i(0, n_reps, 1, hint_engines=tuple(mybir.ALL_ENGINES))
            )
        with (
            tc.tile_pool(name="const", bufs=1) as cpool,
            tc.tile_pool(name="x1p", bufs=1) as x1pool,
        ):
            o = 0
            t_a2lt = cpool.tile([hin, hs], DEV_DT)
            nc.sync.dma_start(
                out=t_a2lt[:],
                in_=consts[o : o + hin * hs].rearrange("(p f) -> p f", p=hin),
            )
            o += hin * hs

            t_x1 = x1pool.tile([128, NJ, BC, hs], DEV_DT)

            # ---------------- pass H (DMA-streamed groups) ----------------
            # First load is a single 8-bc group so the PE starts early; the
            # rest stream LDG groups per DMA. a3t/mbct (only needed by pass
            # W/BC) load after the first x slab is in flight.
            load_plan = [1]
            while sum(load_plan) < n_grp:
                load_plan.append(min(LDG, n_grp - sum(load_plan)))
            with (
                tc.tile_pool(name="ld", bufs=3) as ldpool,
                tc.tile_pool(name="psh", bufs=4, space="PSUM") as ph,
            ):
                g0 = 0
                t_a3t = t_mbct = None
                for li, ng in enumerate(load_plan):
                    xt = ldpool.tile([hin, LDG, GRP, W], DEV_DT, tag="xt")
                    nc.sync.dma_start(
                        out=xt[:, :ng],
                        in_=xs[:, g0 * GRP : (g0 + ng) * GRP, :].rearrange(
                            "h (l g) w -> h l g w", l=ng
                        ),
                    )
                    if li == 0:
                        t_a3t = cpool.tile([128, NJ, W], DEV_DT)
                        nc.sync.dma_start(
                            out=t_a3t[:],
                            in_=consts[o : o + 128 * NJ * W].rearrange(
                                "(p j n) -> p j n", p=128, j=NJ
                            ),
                        )
                        o += 128 * NJ * W
                        t_mbct = cpool.tile([BC, BCP], DEV_DT)
                        nc.sync.dma_start(
                            out=t_mbct[:],
                            in_=consts[o : o + BC * BCP].rearrange(
                                "(p f) -> p f", p=BC
                            ),
                        )
                    for gg in range(ng):
                        g = g0 + gg
                        for jp in range(NJ // 2):
                            ps = ph.tile([128, 2, GRP * hs], F32)  # 2 banks
                            for jj in range(2):
                                j = 2 * jp + jj
                                for i in range(GRP):
                                    nc.tensor.matmul(
                                        ps[:, jj, i * hs : (i + 1) * hs],
                                        lhsT=xt[:, gg, i, 128 * j : 128 * (j + 1)],
                                        rhs=t_a2lt[:],
                                        start=True,
                                        stop=True,
                                    )
                            dst = t_x1[
                                :, 2 * jp : 2 * jp + 2, g * GRP : (g + 1) * GRP, :
                            ]
                            if (g * NJ // 2 + jp) % 2 == 0:
                                nc.vector.tensor_copy(dst, ps[:])
                            else:
                                nc.scalar.copy(dst, ps[:])
                    g0 += ng

            # ------------- pass W + BC (fused, h-paired, skewed) ----------
            with (
                tc.tile_pool(name="x2p", bufs=3) as x2pool,
                tc.tile_pool(name="stg", bufs=2) as stpool,
                tc.tile_pool(name="psw", bufs=2, space="PSUM") as pw,
                tc.tile_pool(name="psb", bufs=2, space="PSUM") as pb,
            ):
                x2_tiles = {}
                stage = None

                def emit_w_pair(hp):
                    ps = pw.tile([BC, 2, W], F32, tag="w")  # 2 banks
                    for k in range(2):
                        h = hp + k
                        for j in range(NJ):
                            lo, hi = band(j)
                            nc.tensor.matmul(
                                ps[:, k, lo:hi],
                                lhsT=t_x1[:, j, :, h],
                                rhs=t_a3t[:, j, lo:hi],
                                start=(j == 0),
                                stop=(j == NJ - 1),
                            )
                    x2 = x2pool.tile([BC, 2, W], DEV_DT, tag="x2")
                    if (hp // 2) % 2 == 0:
                        nc.vector.tensor_copy(x2[:], ps[:])
                    else:
                        nc.scalar.copy(x2[:], ps[:])
                    x2_tiles[hp] = x2

                def emit_bc_pair(hp):
                    nonlocal stage
                    x2 = x2_tiles.pop(hp)
                    ps2 = pb.tile([BCP, 2, W], F32, tag="b")  # 2 banks
                    for k in range(2):
                        nc.tensor.matmul(
                            ps2[:, k, :],
                            lhsT=t_mbct[:],
                            rhs=x2[:, k, :],
                            start=True,
                            stop=True,
                        )
                    hb, hr = divmod(hp, HB)
                    if hr == 0:
                        stage = stpool.tile([BC, HB, W], DEV_DT, tag="s")
                    if (hp // 2) % 2 == 0:
                        nc.scalar.copy(stage[:, hr : hr + 2, :], ps2[:BC])
                    else:
                        nc.vector.tensor_copy(stage[:, hr : hr + 2, :], ps2[:BC])
                    if hr == HB - 2:
                        # scalar-engine HWDGE ring: keeps output stores off
                        # the sync ring that streams the input loads
                        nc.sync.dma_start(
                            out=out[:, hb * HB : (hb + 1) * HB, :], in_=stage[:]
                        )

                SKEW = 4  # h units (2 pairs)
                for hp in range(0, hs, 2):
                    emit_w_pair(hp)
                    if hp >= SKEW:
                        emit_bc_pair(hp - SKEW)
                for hp in range(hs - SKEW, hs, 2):
                    emit_bc_pair(hp)
    nc.finalize()
    return nc


_CACHE = {}


def build_program_for(x_shape, k_size, n_reps=1):
    """Program with the same I/O contract as prepare()'s, optionally looped
    n_reps times on-device (timing use)."""
    B, C, H, W = x_shape
    w, radius = _gauss_kernel(float(k_size))
    hs = H // N_CORES
    hin = hs + 2 * radius
    return _build_program(B, C, H, W, hin, hs, radius, n_reps=n_reps)


def prepare(x, k_size):
    """Build (cached) program + per-core input maps for the given x."""
    x = np.ascontiguousarray(np.asarray(x, dtype=np.float32))
    B, C, H, W = x.shape
    BC = B * C
    sigma = float(k_size)
    w, radius = _gauss_kernel(sigma)

    hs = H // N_CORES
    hin = hs + 2 * radius
    assert hin <= min(H, 128), (hin, H)

    key = (B, C, H, W, sigma)
    if key not in _CACHE:
        A0 = _blur_matrix(B, w, radius)
        A1 = _blur_matrix(C, w, radius)
        A2 = _blur_matrix(H, w, radius)
        A3 = _blur_matrix(W, w, radius)

        # band-structure sanity: chunk j' of A3^T only reaches cols [lo, hi)
        A3T = A3.T
        for j in range(W // 128):
            lo, hi = max(0, 128 * j - radius), min(W, 128 * j + 128 + radius)
            assert np.abs(np.delete(A3T[128 * j : 128 * (j + 1)], np.s_[lo:hi], axis=1)).max() == 0.0

        # a3t in device layout [128, NJ, W] (partition-major, contiguous DMA)
        a3tp = np.ascontiguousarray(
            A3T.reshape(W // 128, 128, W).transpose(1, 0, 2).astype(DEV_NP)
        )
        mbct = np.kron(A0, A1).T.astype(DEV_NP)  # [bc_in 96, bc_out 96]
        mbct_pad = np.zeros((BC, 128), DEV_NP)
        mbct_pad[:, :BC] = mbct

        h0s, a2lts = [], []
        for m in range(N_CORES):
            h0 = min(max(hs * m - radius, 0), H - hin)
            rows = A2[hs * m : hs * (m + 1), :]
            mask = np.ones(H, bool)
            mask[h0 : h0 + hin] = False
            assert np.abs(rows[:, mask]).max() == 0.0, m
            h0s.append(h0)
            a2lts.append(
                np.ascontiguousarray(rows[:, h0 : h0 + hin].T.astype(DEV_NP))
            )

        consts = [
            np.concatenate(
                [a2lts[m].ravel(), a3tp.ravel(), mbct_pad.ravel()]
            ).astype(DEV_NP)
            for m in range(N_CORES)
        ]
        nc = _build_program(B, C, H, W, hin, hs, radius)
        _CACHE[key] = (nc, h0s, consts)

    nc, h0s, consts = _CACHE[key]

    x16 = x.astype(DEV_NP)
    in_maps = [
        {
            # [hin, BC, W]: h-slab, transposed so DMA lines are contiguous
            "xs": np.ascontiguousarray(
                x16[:, :, h0s[m] : h0s[m] + hin, :]
                .reshape(BC, hin, W)
                .transpose(1, 0, 2)
            ),
            "consts": consts[m],
        }
        for m in range(N_CORES)
    ]
    return nc, in_maps


def assemble(outs, B=32, C=3, H=512, W=512):
    """Per-core out tensors [BC, hs, W] (fp16) -> full [B, C, H, W] f32."""
    full = np.concatenate(outs, axis=1)
    return np.ascontiguousarray(full.astype(np.float32).reshape(B, C, H, W))


def kernel(x, k_size):
    x = np.ascontiguousarray(np.asarray(x, dtype=np.float32))
    B, C, H, W = x.shape
    nc, in_maps = prepare(x, k_size)
    res = run_bass_kernel_spmd(nc, in_maps, core_ids=list(range(N_CORES)))
    return assemble(
        [res.results[m]["out"] for m in range(N_CORES)], B, C, H, W
    )


# revision 3
# speedup vs baseline: 1.0018x; 1.0018x over previous
"""Trainium2 Bass kernel for nn_BlurTensor: gaussian_filter(sigma=k_size) over
ALL axes of x (B=32, C=3, H=512, W=512) with 'symmetric' boundary.

Decomposition: the blur is the linear operator A0 (x) A1 (x) A2 (x) A3 applied
as mode products (one blur matrix per axis, built on host from k_size).
B and C fold into a single 96x96 Kronecker-product mixing matrix (96 <= 128
partitions), so the device does three matmul passes: H, W (banded), BC.

Sharding: H axis split into 8 x 64-row output slabs; each core receives a
104-row input slab (radius-20 halo), which makes all passes core-local.

v2 (all-fp16 device pipeline, PSUM accumulation stays fp32):
  - fp16 matmuls stream 1 cycle/row vs fp32's 4 (fp32 = 2 half-speed passes);
    fp16 input halves the dominant DMA traffic; device output is fp16 too
    (host upcasts) halving output DMA. End-to-end rel err ~6e-4 vs the 2e-2
    gate.
  - One merged consts tensor (fewer per-call PJRT args); x pre-transposed on
    host to [hin, B*C, W] so every input DMA line is contiguous.
  - PSUM evacuations paired (two banks per copy) to halve the per-op DVE/ACT
    fixed cost; copies alternate between Vector and Scalar engines.

Device pipeline per core (all intermediates SBUF-resident):
  pass H : out[w, (bc,h)] via lhsT = data tile [h'=104, w-chunk 128],
           rhs = A2_local^T [104, 64]  -> X1 [P:w(4x128), F:(wc,bc,h)]
  pass W : banded 512x512 matrix, 4 chunked matmuls accumulating into one
           PSUM bank per h (start=True on first clears has_written; the
           overlapping band writes then accumulate-or-overwrite per element)
  pass BC: Kronecker matrix (padded to 128 cols for fast-weight-load) as
           stationary weights, rhs = X2h [96, 512]
"""

import numpy as np

import concourse.bass as bass
import concourse.bacc as bacc
import concourse.mybir as mybir
from concourse.tile import TileContext
from concourse.bass_utils import run_bass_kernel_spmd

TRUNCATE = 4.0
N_CORES = 8
F32 = mybir.dt.float32
DEV_DT = mybir.dt.float16
DEV_NP = np.float16


def _gauss_kernel(sigma):
    # matches scipy/jax _gaussian_kernel1d in float32, like the reference
    radius = int(TRUNCATE * sigma + 0.5)
    x = np.arange(-radius, radius + 1, dtype=np.float32)
    w = np.exp(-0.5 * (x / sigma) ** 2).astype(np.float32)
    w = (w / w.sum(dtype=np.float32)).astype(np.float32)
    return w.astype(np.float64), radius


def _blur_matrix(L, w, radius):
    """(L, L) float64 operator: blur(v) = A @ v with symmetric padding."""
    I = np.eye(L, dtype=np.float64)
    Ipad = np.pad(I, ((radius, radius), (0, 0)), mode="symmetric")
    A = np.empty((L, L), dtype=np.float64)
    for i in range(L):
        A[i, :] = w @ Ipad[i : i + 2 * radius + 1, :]
    return A


def _build_program(B, C, H, W, hin, hs, radius, n_reps=1):
    """Build the SPMD Bass program (identical on all cores).

    n_reps > 1 wraps the whole pipeline in a device-side For_i that re-runs
    it n_reps times (same DRAM buffers). Used only for timing: the axon PJRT
    dispatch costs ~3.3 ms per call, so per-exec HW time is measured as the
    slope between two rep counts."""
    from contextlib import ExitStack

    BC = B * C
    assert BC <= 128 and hin <= 128 and W % 128 == 0
    NJ = W // 128  # w' chunks
    BCP = 128  # mbct padded cols (bc_out) for fast-weight-load

    n_c = hin * hs + 128 * NJ * W + BC * BCP

    nc = bacc.Bacc("TRN2")
    xs = nc.dram_tensor("xs", [hin, BC, W], DEV_DT, kind="ExternalInput")
    consts = nc.dram_tensor("consts", [n_c], DEV_DT, kind="ExternalInput")
    out = nc.dram_tensor("out", [BC, hs, W], DEV_DT, kind="ExternalOutput")

    GRP = 8  # bc per psum/copy group
    LDG = 2  # load groups per DMA
    n_grp = BC // GRP
    HB = 8  # h rows per output stage/DMA

    def band(j):
        return max(0, 128 * j - radius), min(W, 128 * j + 128 + radius)

    with TileContext(nc) as tc, ExitStack() as _st:
        if n_reps > 1:
            _st.enter_context(
                tc.For_i(0, n_reps, 1, hint_engines=tuple(mybir.ALL_ENGINES))
            )
        with (
            tc.tile_pool(name="const", bufs=1) as cpool,
            tc.tile_pool(name="x1p", bufs=1) as x1pool,
        ):
            o = 0
            t_a2lt = cpool.tile([hin, hs], DEV_DT)
            nc.scalar.dma_start(
                out=t_a2lt[:],
                in_=consts[o : o + hin * hs].rearrange("(p f) -> p f", p=hin),
            )
            o += hin * hs

            t_x1 = x1pool.tile([128, NJ, BC, hs], DEV_DT)

            # ---------------- pass H (DMA-streamed groups) ----------------
            # First load is a single 8-bc group so the PE starts early; the
            # rest stream LDG groups per DMA. a3t/mbct (only needed by pass
            # W/BC) load after the first x slab is in flight.
            load_plan = [1]
            while sum(load_plan) < n_grp:
                load_plan.append(min(LDG, n_grp - sum(load_plan)))
            with (
                tc.tile_pool(name="ld", bufs=3) as ldpool,
                tc.tile_pool(name="psh", bufs=4, space="PSUM") as ph,
            ):
                g0 = 0
                t_a3t = t_mbct = None
                for li, ng in enumerate(load_plan):
                    xt = ldpool.tile([hin, LDG, GRP, W], DEV_DT, tag="xt")
                    nc.sync.dma_start(
                        out=xt[:, :ng],
                        in_=xs[:, g0 * GRP : (g0 + ng) * GRP, :].rearrange(
                            "h (l g) w -> h l g w", l=ng
                        ),
                    )
                    if li == 0:
                        t_a3t = cpool.tile([128, NJ, W], DEV_DT)
                        nc.scalar.dma_start(
                            out=t_a3t[:],
                            in_=consts[o : o + 128 * NJ * W].rearrange(
                                "(p j n) -> p j n", p=128, j=NJ
                            ),
                        )
                        o += 128 * NJ * W
                        t_mbct = cpool.tile([BC, BCP], DEV_DT)
                        nc.scalar.dma_start(
                            out=t_mbct[:],
                            in_=consts[o : o + BC * BCP].rearrange(
                                "(p f) -> p f", p=BC
                            ),
                        )
                    for gg in range(ng):
                        g = g0 + gg
                        for jp in range(NJ // 2):
                            ps = ph.tile([128, 2, GRP * hs], F32)  # 2 banks
                            for jj in range(2):
                                j = 2 * jp + jj
                                for i in range(GRP):
                                    nc.tensor.matmul(
                                        ps[:, jj, i * hs : (i + 1) * hs],
                                        lhsT=xt[:, gg, i, 128 * j : 128 * (j + 1)],
                                        rhs=t_a2lt[:],
                                        start=True,
                                        stop=True,
                                    )
                            dst = t_x1[
                                :, 2 * jp : 2 * jp + 2, g * GRP : (g + 1) * GRP, :
                            ]
                            if (g * NJ // 2 + jp) % 2 == 0:
                                nc.vector.tensor_copy(dst, ps[:])
                            else:
                                nc.scalar.copy(dst, ps[:])
                    g0 += ng

            # ------------- pass W + BC (fused, h-paired, skewed) ----------
            with (
                tc.tile_pool(name="x2p", bufs=3) as x2pool,
                tc.tile_pool(name="stg", bufs=2) as stpool,
                tc.tile_pool(name="psw", bufs=2, space="PSUM") as pw,
                tc.tile_pool(name="psb", bufs=2, space="PSUM") as pb,
            ):
                x2_tiles = {}
                stage = None

                def emit_w_pair(hp):
                    ps = pw.tile([BC, 2, W], F32, tag="w")  # 2 banks
                    for k in range(2):
                        h = hp + k
                        for j in range(NJ):
                            lo, hi = band(j)
                            nc.tensor.matmul(
                                ps[:, k, lo:hi],
                                lhsT=t_x1[:, j, :, h],
                                rhs=t_a3t[:, j, lo:hi],
                                start=(j == 0),
                                stop=(j == NJ - 1),
                            )
                    x2 = x2pool.tile([BC, 2, W], DEV_DT, tag="x2")
                    if (hp // 2) % 2 == 0:
                        nc.vector.tensor_copy(x2[:], ps[:])
                    else:
                        nc.scalar.copy(x2[:], ps[:])
                    x2_tiles[hp] = x2

                def emit_bc_pair(hp):
                    nonlocal stage
                    x2 = x2_tiles.pop(hp)
                    ps2 = pb.tile([BCP, 2, W], F32, tag="b")  # 2 banks
                    for k in range(2):
                        nc.tensor.matmul(
                            ps2[:, k, :],
                            lhsT=t_mbct[:],
                            rhs=x2[:, k, :],
                            start=True,
                            stop=True,
                        )
                    hb, hr = divmod(hp, HB)
                    if hr == 0:
                        stage = stpool.tile([BC, HB, W], DEV_DT, tag="s")
                    if (hp // 2) % 2 == 0:
                        nc.scalar.copy(stage[:, hr : hr + 2, :], ps2[:BC])
                    else:
                        nc.vector.tensor_copy(stage[:, hr : hr + 2, :], ps2[:BC])
                    if hr == HB - 2:
                        # scalar-engine HWDGE ring: keeps output stores off
                        # the sync ring that streams the input loads
                        nc.sync.dma_start(
                            out=out[:, hb * HB : (hb + 1) * HB, :], in_=stage[:]
                        )

                SKEW = 4  # h units (2 pairs)
                for hp in range(0, hs, 2):
                    emit_w_pair(hp)
                    if hp >= SKEW:
                        emit_bc_pair(hp - SKEW)
                for hp in range(hs - SKEW, hs, 2):
                    emit_bc_pair(hp)
    nc.finalize()
    return nc


_CACHE = {}


def build_program_for(x_shape, k_size, n_reps=1):
    """Program with the same I/O contract as prepare()'s, optionally looped
    n_reps times on-device (timing use)."""
    B, C, H, W = x_shape
    w, radius = _gauss_kernel(float(k_size))
    hs = H // N_CORES
    hin = hs + 2 * radius
    return _build_program(B, C, H, W, hin, hs, radius, n_reps=n_reps)


def prepare(x, k_size):
    """Build (cached) program + per-core input maps for the given x."""
    x = np.ascontiguousarray(np.asarray(x, dtype=np.float32))
    B, C, H, W = x.shape
    BC = B * C
    sigma = float(k_size)
    w, radius = _gauss_kernel(sigma)

    hs = H // N_CORES
    hin = hs + 2 * radius
    assert hin <= min(H, 128), (hin, H)

    key = (B, C, H, W, sigma)
    if key not in _CACHE:
        A0 = _blur_matrix(B, w, radius)
        A1 = _blur_matrix(C, w, radius)
        A2 = _blur_matrix(H, w, radius)
        A3 = _blur_matrix(W, w, radius)

        # band-structure sanity: chunk j' of A3^T only reaches cols [lo, hi)
        A3T = A3.T
        for j in range(W // 128):
            lo, hi = max(0, 128 * j - radius), min(W, 128 * j + 128 + radius)
            assert np.abs(np.delete(A3T[128 * j : 128 * (j + 1)], np.s_[lo:hi], axis=1)).max() == 0.0

        # a3t in device layout [128, NJ, W] (partition-major, contiguous DMA)
        a3tp = np.ascontiguousarray(
            A3T.reshape(W // 128, 128, W).transpose(1, 0, 2).astype(DEV_NP)
        )
        mbct = np.kron(A0, A1).T.astype(DEV_NP)  # [bc_in 96, bc_out 96]
        mbct_pad = np.zeros((BC, 128), DEV_NP)
        mbct_pad[:, :BC] = mbct

        h0s, a2lts = [], []
        for m in range(N_CORES):
            h0 = min(max(hs * m - radius, 0), H - hin)
            rows = A2[hs * m : hs * (m + 1), :]
            mask = np.ones(H, bool)
            mask[h0 : h0 + hin] = False
            assert np.abs(rows[:, mask]).max() == 0.0, m
            h0s.append(h0)
            a2lts.append(
                np.ascontiguousarray(rows[:, h0 : h0 + hin].T.astype(DEV_NP))
            )

        consts = [
            np.concatenate(
                [a2lts[m].ravel(), a3tp.ravel(), mbct_pad.ravel()]
            ).astype(DEV_NP)
            for m in range(N_CORES)
        ]
        nc = _build_program(B, C, H, W, hin, hs, radius)
        _CACHE[key] = (nc, h0s, consts)

    nc, h0s, consts = _CACHE[key]

    x16 = x.astype(DEV_NP)
    in_maps = [
        {
            # [hin, BC, W]: h-slab, transposed so DMA lines are contiguous
            "xs": np.ascontiguousarray(
                x16[:, :, h0s[m] : h0s[m] + hin, :]
                .reshape(BC, hin, W)
                .transpose(1, 0, 2)
            ),
            "consts": consts[m],
        }
        for m in range(N_CORES)
    ]
    return nc, in_maps


def assemble(outs, B=32, C=3, H=512, W=512):
    """Per-core out tensors [BC, hs, W] (fp16) -> full [B, C, H, W] f32."""
    full = np.concatenate(outs, axis=1)
    return np.ascontiguousarray(full.astype(np.float32).reshape(B, C, H, W))


def kernel(x, k_size):
    x = np.ascontiguousarray(np.asarray(x, dtype=np.float32))
    B, C, H, W = x.shape
    nc, in_maps = prepare(x, k_size)
    res = run_bass_kernel_spmd(nc, in_maps, core_ids=list(range(N_CORES)))
    return assemble(
        [res.results[m]["out"] for m in range(N_CORES)], B, C, H, W
    )
